# revision 1
# baseline (speedup 1.0000x reference)
"""Trainium2 Bass kernel for nn_MeshDeformationBlock (GNN message passing).

Data-parallel over batch: 2 batches per core, 8 cores.  Math rewrite:
  out = g@P0 + (A g)@P1 + (A^2 g)@P2 + (A^3 g)@P3      (biases are zero)
with g = bilinear(img, pos) + vertex_padded, A the symmetric edge operator,
P0..P3 host-precomputed 128x128 weight products.  Device work: Ant dma_gather
(bf16, 512B two-batch-interleaved rows, 4 SWDGE queues) + DVE plane adds via
degree-(d,a) subclass chunking; final combine = PE matmuls on xbar-transposed
reads, fp32 PSUM.
"""

import sys
import numpy as np
import ml_dtypes

sys.path.insert(0, "/opt/trn_rl_repo")

bf16 = ml_dtypes.bfloat16

B, V, C, H, W = 16, 40000, 128, 56, 56
NCORES = 8
NB = 2
TOKCAP = 4096
CVB = 2048
MIN_GROUP = 256
NPIX = 3329


# ---------------------------------------------------------------- host plan

def _build_graph_plan(edges):
    e = np.asarray(edges).astype(np.int64)
    src = np.concatenate([e[:, 1], e[:, 0]])
    dst = np.concatenate([e[:, 0], e[:, 1]])
    deg = np.bincount(dst, minlength=V).astype(np.int64)

    order = np.argsort(dst, kind="stable")
    nbr_flat = src[order]
    rowptr = np.zeros(V + 1, np.int64)
    rowptr[1:] = np.cumsum(deg)

    dmax = int(deg.max())
    counts_by_d = np.bincount(deg, minlength=dmax + 1)
    cum = np.cumsum(counts_by_d)
    dstar = int(np.searchsorted(cum, 18000))
    halfbit = deg <= dstar

    nbr_low = halfbit[nbr_flat]
    a_of = np.zeros(V, np.int64)
    np.add.at(a_of, dst[order], nbr_low.astype(np.int64))

    keys = {}
    dd, aa = deg, a_of
    for v in range(V):
        keys.setdefault((int(dd[v]), int(aa[v])), []).append(v)
    low_keys = sorted(k for k in keys if k[0] <= dstar)
    high_keys = sorted(k for k in keys if k[0] > dstar)

    def merge(klist):
        groups, cur, ca, cb = [], [], 0, 0
        for (d, a) in klist:
            cur.extend(keys[(d, a)])
            ca = max(ca, a)
            cb = max(cb, d - a)
            if len(cur) >= MIN_GROUP:
                groups.append((cur, ca, cb))
                cur, ca, cb = [], 0, 0
        if cur:
            groups.append((cur, ca, cb))
        return groups

    low_groups = [([], 0, 0)] + merge(low_keys)
    high_groups = merge(high_keys) + [([], 0, 0)]

    rowpos = np.full(V, -1, np.int64)
    group_meta = []
    pos = 0
    half_rows = None
    for side, groups in ((0, low_groups), (1, high_groups)):
        if side == 1:
            half_rows = pos
        for vs, A, Bn in groups:
            nreal = len(vs)
            nslots = max(128, -(-max(nreal, 1) // 128) * 128)
            if nreal:
                rowpos[np.array(vs, np.int64)] = pos + np.arange(nreal)
            group_meta.append((pos, nslots, A, Bn))
            pos += nslots
    Vp = -(-pos // 512) * 512
    if Vp > pos:
        group_meta.append((pos, Vp - pos, 0, 0))
    assert half_rows is not None
    assert half_rows < 32768 and (Vp - half_rows) < 32768, (half_rows, Vp)

    ZL, ZH = 0, Vp - 1
    vert_at = np.full(Vp, -1, np.int64)
    vert_at[rowpos[rowpos >= 0] if False else rowpos] = np.arange(V)

    tok_stream = []
    chunks = []
    off = 0
    for base, nslots, A, Bn in group_meta:
        D = A + Bn
        if D == 0:
            chunks.append((base, nslots, 0, 0, 0, 0))
            continue
        gv = max(128, (TOKCAP // D) // 128 * 128)
        for cb in range(base, base + nslots, gv):
            cg = min(gv, base + nslots - cb)
            lo = np.full((A, cg), ZL, np.int64)
            hi = np.full((Bn, cg), ZH - half_rows, np.int64)
            for u in range(cg):
                v = vert_at[cb + u]
                if v < 0:
                    continue
                ns = nbr_flat[rowptr[v]:rowptr[v + 1]]
                rp = rowpos[ns]
                rlo = rp[rp < half_rows]
                rhi = rp[rp >= half_rows] - half_rows
                lo[: len(rlo), u] = rlo
                hi[: len(rhi), u] = rhi
            off_lo = off
            tok_stream.append(lo.reshape(-1))
            off += A * cg
            off_hi = off
            tok_stream.append(hi.reshape(-1))
            off += Bn * cg
            chunks.append((cb, cg, A, Bn, off_lo, off_hi))
    # storage map: within each chunk rows are p-major (SBUF partition-contiguous)
    log2store = np.empty(Vp, np.int64)
    for (cb, cg, A, Bn, _ol, _oh) in chunks:
        nvb = cg // 128
        q = np.arange(cg)
        log2store[cb + q] = cb + (q % 128) * nvb + q // 128
    tok_l = (np.concatenate(tok_stream) if tok_stream else np.zeros(0, np.int64))
    # token values are logical rows (low: absolute; high: logical-HALF).
    # remap to storage rows.
    # rebuild with absolute logical values to remap, then re-split by half.
    tok_abs = []
    off2 = 0
    for (cb, cg, A, Bn, off_lo, off_hi) in chunks:
        if A + Bn == 0:
            continue
        tok_abs.append(log2store[tok_l[off_lo:off_lo + A * cg]])
        tok_abs.append(log2store[tok_l[off_hi:off_hi + Bn * cg] + half_rows] - half_rows)
    tok = (np.concatenate(tok_abs) if tok_abs else np.zeros(0, np.int64)).astype(np.int16)
    if len(tok) % 128:
        tok = np.concatenate([tok, np.zeros(128 - len(tok) % 128, np.int16)])

    return dict(rowpos=rowpos, vert_at=vert_at, Vp=Vp, half=half_rows,
                chunks=chunks, tok=tok, log2store=log2store)


def _wrap16(stream):
    n = len(stream)
    assert n % 16 == 0
    w = stream.reshape(n // 16, 16).T
    return np.ascontiguousarray(np.tile(w, (8, 1))).astype(np.int16)


def _bilinear_host(plan, pos_b):
    Vp = plan["Vp"]
    vert_at = plan["vert_at"]
    x = (pos_b[:, 0] + 1.0) * 0.5 * (W - 1)
    y = (pos_b[:, 1] + 1.0) * 0.5 * (H - 1)
    x0 = np.floor(x)
    y0 = np.floor(y)
    wx1 = (x - x0).astype(np.float32)
    wx0 = 1.0 - wx1
    wy1 = (y - y0).astype(np.float32)
    wy0 = 1.0 - wy1
    x0 = np.clip(x0.astype(np.int64), 0, W - 1)
    y0 = np.clip(y0.astype(np.int64), 0, H - 1)

    idxA = np.zeros(Vp, np.int64)
    idxB = np.zeros(Vp, np.int64)
    wA = np.zeros((Vp, 2), np.float32)
    wB = np.zeros((Vp, 2), np.float32)
    mask = vert_at >= 0
    vs = vert_at[mask]
    idxA[mask] = y0[vs] * W + x0[vs]
    idxB[mask] = np.minimum(y0[vs] + 1, H - 1) * W + x0[vs]
    wA[mask, 0] = wx0[vs] * wy0[vs]
    wA[mask, 1] = wx1[vs] * wy0[vs]
    wB[mask, 0] = wx0[vs] * wy1[vs]
    wB[mask, 1] = wx1[vs] * wy1[vs]

    # permute logical->storage, then emit gather streams/weights so that
    # phase-B slot (p,u) of block r0 holds the vertex at storage row r0+p*16+u:
    # the bilinear gather writes slot (p,u) from token position u*128+p, so the
    # token stream must be in colmajor order of the storage-blocked layout.
    l2s = plan["log2store"]
    idxA_s = np.zeros(Vp, np.int64); idxA_s[l2s] = idxA
    idxB_s = np.zeros(Vp, np.int64); idxB_s[l2s] = idxB
    wA_s = np.zeros((Vp, 2), np.float32); wA_s[l2s] = wA
    wB_s = np.zeros((Vp, 2), np.float32); wB_s[l2s] = wB
    stream = []
    for r0 in range(0, Vp, CVB):
        cv = min(CVB, Vp - r0)
        nv = cv // 128
        t = np.arange(cv)
        rows = r0 + (t % 128) * nv + t // 128
        stream.append(idxA_s[rows])
        stream.append(idxB_s[rows])
    stream = np.concatenate(stream).astype(np.int16)
    wAe = np.zeros((Vp, 2, 128), bf16)
    wBe = np.zeros((Vp, 2, 128), bf16)
    wAe[:] = wA_s.astype(bf16)[:, :, None]
    wBe[:] = wB_s.astype(bf16)[:, :, None]
    return _wrap16(stream), wAe.reshape(Vp, 256), wBe.reshape(Vp, 256)


# ---------------------------------------------------------------- device

def _build_kernel(plan):
    import concourse.bacc as bacc
    import concourse.mybir as mybir
    from concourse.tile import TileContext

    Vp, half = plan["Vp"], plan["half"]
    chunks = plan["chunks"]
    TOK = len(plan["tok"])

    nc = bacc.Bacc("TRN2", target_bir_lowering=False, debug=False,
                   num_swdge_queues=4)
    dt = mybir.dt

    imgp = nc.dram_tensor("imgp", [NB, NPIX, 256], dt.bfloat16, kind="ExternalInput")
    vpadp = nc.dram_tensor("vpadp", [Vp, 256], dt.bfloat16, kind="ExternalInput")
    bidx = nc.dram_tensor("bidx", [NB, 128, (2 * Vp) // 16], dt.int16, kind="ExternalInput")
    wAe = nc.dram_tensor("wAe", [NB, Vp, 256], dt.bfloat16, kind="ExternalInput")
    wBe = nc.dram_tensor("wBe", [NB, Vp, 256], dt.bfloat16, kind="ExternalInput")
    aidx = nc.dram_tensor("aidx", [128, TOK // 16], dt.int16, kind="ExternalInput")
    Pmat = nc.dram_tensor("Pmat", [4, 128, 128], dt.bfloat16, kind="ExternalInput")
    outcm = nc.dram_tensor("outcm", [NB, 128, Vp], dt.float32, kind="ExternalOutput")

    g_t = nc.dram_tensor("g_t", [Vp, 256], dt.bfloat16)
    a_t = [nc.dram_tensor(f"a{r}_t", [Vp, 256], dt.bfloat16) for r in range(3)]

    def cm(dram_rows):
        return dram_rows.rearrange("(p u) e -> p u e", p=128)

    qn = [0]
    with TileContext(nc) as tc:
        with tc.tile_pool(name="res", bufs=1) as res:
            aidx_sb = res.tile([128, TOK // 16], dt.int16)
            nc.sync.dma_start(out=aidx_sb[:], in_=aidx[:, :])
            P_sb = res.tile([128, 4, 128], dt.bfloat16)
            nc.sync.dma_start(out=P_sb[:], in_=Pmat[:, :, :].rearrange("k p m -> p k m"))
            zt = res.tile([128, 256], dt.bfloat16)
            nc.vector.memset(zt[:], 0.0)

            # ------------- phase B: g = bilinear + vpad -------------
            with tc.tile_pool(name="bil", bufs=2) as bilp:
                bidx_sb = []
                for b in range(NB):
                    t = res.tile([128, (2 * Vp) // 16], dt.int16, tag=f"bi{b}")
                    nc.sync.dma_start(out=t[:], in_=bidx[b, :, :])
                    bidx_sb.append(t)
                for r0 in range(0, Vp, CVB):
                    cv = min(CVB, Vp - r0)
                    nv = cv // 128
                    gst = bilp.tile([128, nv, 2, 128], dt.bfloat16, tag="gst")
                    for b in range(NB):
                        taps = bilp.tile([128, 2 * nv, 256], dt.bfloat16, tag="taps")
                        toff = 2 * r0
                        nc.gpsimd.dma_gather(
                            taps[:], imgp[b, :, :],
                            bidx_sb[b][:, toff // 16:(toff + 2 * cv) // 16],
                            2 * cv, 2 * cv, 256, single_packet=False,
                            queue_num=qn[0] % 4)
                        qn[0] += 1
                        wa = bilp.tile([128, nv, 256], dt.bfloat16, tag="wa")
                        wb = bilp.tile([128, nv, 256], dt.bfloat16, tag="wb")
                        nc.sync.dma_start(out=wa[:], in_=cm(wAe[b, r0:r0 + cv, :]))
                        nc.sync.dma_start(out=wb[:], in_=cm(wBe[b, r0:r0 + cv, :]))
                        vp = bilp.tile([128, nv, 2, 128], dt.bfloat16, tag="vp")
                        nc.sync.dma_start(
                            out=vp[:],
                            in_=cm(vpadp[r0:r0 + cv, :])
                            .rearrange("p u (x c) -> p u x c", x=2))
                        nc.vector.tensor_mul(out=taps[:, :nv, :],
                                             in0=taps[:, :nv, :], in1=wa[:])
                        nc.vector.tensor_mul(out=taps[:, nv:, :],
                                             in0=taps[:, nv:, :], in1=wb[:])
                        nc.vector.tensor_add(out=taps[:, :nv, :],
                                             in0=taps[:, :nv, :],
                                             in1=taps[:, nv:, :])
                        t4 = taps[:, :nv, :].rearrange("p a (x c) -> p a x c", x=2)
                        nc.vector.tensor_add(out=t4[:, :, 0, :],
                                             in0=t4[:, :, 0, :], in1=t4[:, :, 1, :])
                        nc.vector.tensor_add(out=gst[:, :, b, :],
                                             in0=t4[:, :, 0, :], in1=vp[:, :, b, :])
                    nc.sync.dma_start(
                        out=cm(g_t.ap()[r0:r0 + cv, :]),
                        in_=gst[:].rearrange("p u x c -> p u (x c)"))

            # ------------- phase C: a_{r+1} = A a_r -----------------
            with (tc.tile_pool(name="gb", bufs=6) as gbp,
                  tc.tile_pool(name="ac", bufs=4) as acp):
                for r in range(3):
                    src = g_t if r == 0 else a_t[r - 1]
                    dst = a_t[r]
                    for (base, gv, A, Bn, off_lo, off_hi) in chunks:
                        D = A + Bn
                        if D == 0:
                            for u0 in range(0, gv, 128):
                                nc.sync.dma_start(
                                    out=cm(dst.ap()[base + u0:base + u0 + 128, :]),
                                    in_=zt[:].rearrange("p (u e) -> p u e", u=1))
                            continue
                        buf = gbp.tile([128, (gv * D) // 128, 256], dt.bfloat16,
                                       tag="gb")
                        if A:
                            nc.gpsimd.dma_gather(
                                buf[:, :(gv * A) // 128, :], src.ap()[:, :],
                                aidx_sb[:, off_lo // 16:(off_lo + gv * A) // 16],
                                gv * A, gv * A, 256, single_packet=False,
                                queue_num=qn[0] % 4)
                            qn[0] += 1
                        if Bn:
                            nc.gpsimd.dma_gather(
                                buf[:, (gv * A) // 128:, :], src.ap()[half:, :],
                                aidx_sb[:, off_hi // 16:(off_hi + gv * Bn) // 16],
                                gv * Bn, gv * Bn, 256, single_packet=False,
                                queue_num=qn[0] % 4)
                            qn[0] += 1
                        nvb = gv // 128
                        if D == 1:
                            nc.sync.dma_start(out=cm(dst.ap()[base:base + gv, :]),
                                              in_=buf[:, :nvb, :])
                            continue
                        acc = acp.tile([128, nvb, 256], dt.bfloat16, tag="acc")
                        nc.vector.tensor_add(out=acc[:], in0=buf[:, :nvb, :],
                                             in1=buf[:, nvb:2 * nvb, :])
                        for k in range(2, D):
                            nc.vector.tensor_add(
                                out=acc[:], in0=acc[:],
                                in1=buf[:, k * nvb:(k + 1) * nvb, :])
                        nc.sync.dma_start(out=cm(dst.ap()[base:base + gv, :]),
                                          in_=acc[:])

            # ------------- phase D: combine -------------------------
            with (tc.tile_pool(name="dp", bufs=3) as dp,
                  tc.tile_pool(name="ps", bufs=4, space="PSUM") as psp):
                for b in range(NB):
                    for blk in range(0, Vp, 512):
                        ps = psp.tile([128, 512], dt.float32)
                        for k, T in enumerate([g_t, a_t[0], a_t[1], a_t[2]]):
                            xT = dp.tile([128, 512], dt.bfloat16, tag=f"x{k}")
                            nc.sync.dma_start(
                                out=xT[:],
                                in_=T.ap()[blk:blk + 512, :]
                                .rearrange("r (x c) -> r x c", x=2)[:, b, :],
                                transpose=True)
                            nc.tensor.matmul(out=ps[:],
                                             lhsT=P_sb[:, k, :],
                                             rhs=xT[:], start=(k == 0),
                                             stop=(k == 3))
                        ot = dp.tile([128, 512], dt.float32, tag="ot")
                        nc.scalar.activation(
                            out=ot[:], in_=ps[:],
                            func=mybir.ActivationFunctionType.Copy)
                        nc.sync.dma_start(out=outcm[b, :, blk:blk + 512],
                                          in_=ot[:])

    nc.compile()
    return nc


# ---------------------------------------------------------------- entry

def _make_in_maps(plan, inputs):
    Vp = plan["Vp"]
    M = [np.asarray(inputs[f"w0_{i}"], np.float64) for i in (1, 2, 3)]
    N = [np.asarray(inputs[f"w1_{i}"], np.float64) for i in (1, 2, 3)]
    P0 = M[0] + M[0] @ M[1] @ M[2]
    P1 = N[0] + N[0] @ M[1] @ M[2] + M[0] @ N[1] @ M[2] + M[0] @ M[1] @ N[2]
    P2 = N[0] @ N[1] @ M[2] + N[0] @ M[1] @ N[2] + M[0] @ N[1] @ N[2]
    P3 = N[0] @ N[1] @ N[2]
    Pm = np.ascontiguousarray(np.stack([P0, P1, P2, P3]).astype(bf16))

    img = np.asarray(inputs["img_features"], np.float32)
    pos = np.asarray(inputs["vertex_position"], np.float32)
    vpad = np.asarray(inputs["vertex_padded"], np.float32)

    imgr = img.transpose(0, 2, 3, 1).reshape(B, H * W, C).astype(bf16)
    imgpad = np.zeros((B, NPIX + 1, C), bf16)
    imgpad[:, :H * W] = imgr
    imgp_all = np.concatenate([imgpad[:, :NPIX], imgpad[:, 1:NPIX + 1]], axis=2)

    aidx_w = _wrap16(plan["tok"])
    mask = plan["vert_at"] >= 0
    vs = plan["vert_at"][mask]

    in_maps = []
    for core in range(NCORES):
        bs = [NB * core + i for i in range(NB)]
        bidx_l, wAe_l, wBe_l = [], [], []
        for b in bs:
            bi, wa, wb = _bilinear_host(plan, pos[b])
            bidx_l.append(bi)
            wAe_l.append(wa)
            wBe_l.append(wb)
        vpadp = np.zeros((Vp, 2, 128), bf16)
        srows = plan["log2store"][plan["rowpos"]]
        for i, b in enumerate(bs):
            vpadp[srows, i, :] = vpad[b].astype(bf16)
        in_maps.append({
            "imgp": np.ascontiguousarray(np.stack([imgp_all[b] for b in bs])),
            "vpadp": np.ascontiguousarray(vpadp.reshape(Vp, 256)),
            "bidx": np.ascontiguousarray(np.stack(bidx_l)),
            "wAe": np.ascontiguousarray(np.stack(wAe_l)),
            "wBe": np.ascontiguousarray(np.stack(wBe_l)),
            "aidx": aidx_w,
            "Pmat": Pm,
        })
    return in_maps


_CACHE = {}


def kernel(**inputs):
    from concourse import bass_utils

    plan = _build_graph_plan(inputs["edges"])
    in_maps = _make_in_maps(plan, inputs)
    key = "nc"
    if key not in _CACHE:
        _CACHE[key] = _build_kernel(plan)
    nc = _CACHE[key]
    res = bass_utils.run_bass_kernel_spmd(nc, in_maps, core_ids=list(range(NCORES)))

    srows = plan["log2store"][plan["rowpos"]]
    out = np.zeros((B, V, C), np.float32)
    for core in range(NCORES):
        oc = res.results[core]["outcm"]
        for i in range(NB):
            out[NB * core + i] = oc[i][:, srows].T
    return out

